# revision 18
# baseline (speedup 1.0000x reference)
"""CrossAttention Trainium2 Bass kernel (v7b).

Problem (hardcoded): B=16, Lq=Lk=2048, Dq=768, Dk=1024, fp32.
  q = query @ Wq + bq ; k = key @ Wk + bk ; v = key @ Wv + bv
  out = softmax(q k^T / sqrt(1024)) @ v

Sharding: data-parallel over batch, 2 batches per core on 8 cores.

Math simplifications (exact up to rounding):
  - bk shifts every score row by a per-query constant -> cancels in softmax.
  - bv folds into v (softmax weights sum to 1): v' = k@Wv + bv gives
    out = (sum exp * v') / sum exp directly.
  - scores are bounded (|s|/32 < ~3) so exp() without max-subtraction is safe.

v3 (vs v2): all DMAs use host-prepacked SBUF-image layouts (128 fat
contiguous descriptors per transfer instead of ~1024 thin ones), cutting
DMA-issue serialization at kernel start; weight DMAs issue in phase order
(Wk, Wv first); output is downloaded bf16 and cast to fp32 on the host.
Device work per batch: kT = Wk^T keyT and v = keyT^T Wv + bv resident in
SBUF, then per 512-col Lq tile: qT = Wq^T queryT + bq, scoresT = kT^T qT,
exp(s/32), PV + ones-column row sums, out = PV * (1/sums).
"""

import numpy as np

B, LQ, LK = 16, 2048, 2048
DQ, DK = 768, 1024
N_CORES = 8
BPC = B // N_CORES  # batches per core

KCQ = DQ // 128  # 6 contraction chunks for q projection
KCK = DK // 128  # 8 contraction chunks for k/v projection + scores
NLK = LK // 128  # 16 Lk subtiles of 128
C_T = 512
NCQ = LQ // C_T  # Lq tiles
NTK = LK // 512  # Lk tiles (projection phase)


def build_nc(bpc=BPC, lq=LQ, lk=LK, c_t=C_T):
    import concourse.bass as bass
    import concourse.mybir as mybir
    from concourse import bacc
    import concourse.tile as tile

    fp32 = mybir.dt.float32
    bf16 = mybir.dt.bfloat16
    CS = c_t // 128  # Lq subtiles per attention tile

    nc = bacc.Bacc("TRN2")
    # Host-prepacked SBUF-image layouts: partition dim explicit, per-partition
    # slabs contiguous so every DMA is 128 fat descriptors.
    queryT = nc.dram_tensor(
        "queryT", [bpc, NCQ, 128, KCQ, c_t], bf16, kind="ExternalInput")
    keyT = nc.dram_tensor(
        "keyT", [bpc, NTK, 128, KCK, 512], bf16, kind="ExternalInput")
    Wq = nc.dram_tensor("Wq", [128, KCQ, DK], bf16, kind="ExternalInput")
    Wk = nc.dram_tensor("Wk", [128, KCK, DK], bf16, kind="ExternalInput")
    Wv = nc.dram_tensor("Wv", [128, KCK, DK], bf16, kind="ExternalInput")
    bq = nc.dram_tensor("bq", [DK], fp32, kind="ExternalInput")
    bv = nc.dram_tensor("bv", [DK], fp32, kind="ExternalInput")
    out = nc.dram_tensor("out", [bpc, lq, DK], bf16, kind="ExternalOutput")

    fp8 = mybir.dt.float8e4
    DR = mybir.MatmulPerfMode.DoubleRow

    def mm(ps, lhsT, rhs, start, stop, perf_mode=None):
        nc.tensor.matmul(ps, lhsT, rhs, start=start, stop=stop,
                         perf_mode=perf_mode)

    with tile.TileContext(nc) as tc:
        with (
            tc.tile_pool(name="const", bufs=1) as constp,
            tc.tile_pool(name="w", bufs=1) as wp,
            tc.tile_pool(name="kT", bufs=1) as kTp,
            tc.tile_pool(name="v", bufs=1) as vp,
            tc.tile_pool(name="kstage", bufs=2) as ksp,
            tc.tile_pool(name="qstage", bufs=2) as qsp,
            tc.tile_pool(name="qT", bufs=2) as qTp,
            tc.tile_pool(name="exp", bufs=NLK + 2) as ep,
            tc.tile_pool(name="osb", bufs=2) as op,
            tc.tile_pool(name="ps512", bufs=3, space="PSUM") as ps512,
            tc.tile_pool(name="ps_o", bufs=2, space="PSUM") as ps_op,
            tc.tile_pool(name="ps_n", bufs=1, space="PSUM") as ps_np,
        ):
            # Phase-order DMA issue: Wk halves first (first matmuls), then Wv
            # (needed ~10us in), bv (v evacuation), then Wq/bq (phase A).
            # Interleave Wk and the first key tile so the first accumulation
            # group's operands (kc 0-3 of both) land earliest; the kc 4-7
            # halves stream in during the first matmul group.
            wk_sb = wp.tile([128, KCK, DK], bf16)
            kst0 = ksp.tile([128, KCK, 512], bf16, tag="kst")
            nc.sync.dma_start(wk_sb[:, 0:4, :], Wk[:, 0:4, :])
            nc.sync.dma_start(kst0[:, 0:4, :], keyT[0, 0, :, 0:4, :])
            nc.sync.dma_start(wk_sb[:, 4:8, :], Wk[:, 4:8, :])
            nc.sync.dma_start(kst0[:, 4:8, :], keyT[0, 0, :, 4:8, :])
            wv_sb = wp.tile([128, KCK, DK], bf16)
            nc.sync.dma_start(wv_sb, Wv[:, :, :])
            bv_rep = constp.tile([128, DK], fp32)
            nc.sync.dma_start(bv_rep, bv[None, :].partition_broadcast(128))
            wq_sb = wp.tile([128, KCQ, DK], bf16)
            nc.sync.dma_start(wq_sb, Wq[:, :, :])
            bq_sb = constp.tile([128, KCK], fp32)
            nc.sync.dma_start(bq_sb, bq.rearrange("(c p) -> p c", p=128))
            ones_f32 = constp.tile([128, 4], fp32)
            nc.vector.memset(ones_f32, 1.0)
            ones_col = constp.tile([128, 4], bf16)
            nc.vector.tensor_copy(ones_col, ones_f32)
            # HAM pre-warm: tiny dummy matmuls during the initial weight DMA
            # wait keep the PE busy so the clock gate opens (K=8/8) before
            # the real matmuls start.
            for _ in range(64):
                ps = ps512.tile([128, 512], fp32, tag="mm512")
                nc.tensor.matmul(ps[0:4, 0:4], ones_col, ones_col,
                                 start=True, stop=True)

            for b in range(bpc):
                # fp8 scores: kT/qT stored fp8e4m3; score matmuls run as
                # DoubleRow pairs (K=256 per matmul, ~2x PE rate).
                kT8_sb = kTp.tile([128, 6, lk], fp8, tag="kT8")   # [dk0:768, lk]
                kT_sb = kTp.tile([128, 2, lk], bf16, tag="kT")    # [dk768:, lk]
                v_sb = vp.tile([128, NLK, DK], bf16, tag="v")      # [lk, dk]

                # ---- Phase B: kT and v (+bv), SBUF resident ----
                for t in range(NTK):
                    if b == 0 and t == 0:
                        kst = kst0
                    else:
                        kst = ksp.tile([128, KCK, 512], bf16, tag="kst")
                        nc.sync.dma_start(kst[:, 0:4, :], keyT[b, t, :, 0:4, :])
                        nc.sync.dma_start(kst[:, 4:8, :], keyT[b, t, :, 4:8, :])
                    for mc in range(KCK):
                        ps = ps512.tile([128, 512], fp32, tag="mm512")
                        for kc in range(KCK):
                            mm(ps, wk_sb[:, kc, mc * 128:(mc + 1) * 128],
                               kst[:, kc, :], kc == 0, kc == KCK - 1)
                        if mc < 6:
                            nc.vector.tensor_copy(
                                kT8_sb[:, mc, t * 512:(t + 1) * 512], ps
                            )
                        else:
                            nc.vector.tensor_copy(
                                kT_sb[:, mc - 6, t * 512:(t + 1) * 512], ps
                            )
                    for s in range(4):
                        for dk in range(2):
                            ps = ps_op.tile([128, DK], fp32, tag="pv")
                            for kc in range(KCK):
                                mm(ps[:, 0:512],
                                   kst[:, kc, s * 128:(s + 1) * 128],
                                   wv_sb[:, kc, dk * 512:(dk + 1) * 512],
                                   kc == 0, kc == KCK - 1)
                            nc.vector.tensor_add(
                                v_sb[:, t * 4 + s, dk * 512:(dk + 1) * 512],
                                ps[:, 0:512],
                                bv_rep[:, dk * 512:(dk + 1) * 512],
                            )

                # ---- Phase A+C fused per Lq tile ----
                for t in range(NCQ):
                    qst = qsp.tile([128, KCQ, c_t], bf16, tag="qst")
                    nc.sync.dma_start(qst, queryT[b, t])
                    qT8 = qTp.tile([128, 6, c_t], fp8, tag="qT8")
                    qTt = qTp.tile([128, 2, c_t], bf16, tag="qTt")
                    for mc in range(KCK):
                        ps = ps512.tile([128, c_t], fp32, tag="mm512")
                        for kc in range(KCQ):
                            mm(ps, wq_sb[:, kc, mc * 128:(mc + 1) * 128],
                               qst[:, kc, :], kc == 0, kc == KCQ - 1)
                        if mc < 6:
                            nc.vector.tensor_scalar_add(
                                qT8[:, mc, :], ps, bq_sb[:, mc:mc + 1]
                            )
                        else:
                            nc.vector.tensor_scalar_add(
                                qTt[:, mc - 6, :], ps, bq_sb[:, mc:mc + 1]
                            )
                    exps = []
                    for lkb in range(NLK):
                        ps_s = ps512.tile([128, c_t], fp32, tag="mm512")
                        lks = slice(lkb * 128, (lkb + 1) * 128)
                        for j in range(3):
                            mm(ps_s, kT8_sb[:, 2 * j:2 * j + 2, lks],
                               qT8[:, 2 * j:2 * j + 2, :],
                               j == 0, False, perf_mode=DR)
                        for kc in range(2):
                            mm(ps_s, kT_sb[:, kc, lks],
                               qTt[:, kc, :], False, kc == 1)
                        ex = ep.tile([128, c_t], bf16, tag="exp")
                        nc.scalar.activation(
                            ex, ps_s, mybir.ActivationFunctionType.Exp,
                            scale=1.0 / 32.0,
                        )
                        exps.append(ex)
                    for s in range(CS):
                        ps_o = ps_op.tile([128, DK], fp32, tag="pv")
                        ps_n = ps_np.tile([128, 4], fp32, tag="sum")
                        for lkb in range(NLK):
                            lhs = exps[lkb][:, s * 128:(s + 1) * 128]
                            for dk in range(2):
                                mm(ps_o[:, dk * 512:(dk + 1) * 512], lhs,
                                   v_sb[:, lkb, dk * 512:(dk + 1) * 512],
                                   lkb == 0, lkb == NLK - 1)
                            mm(ps_n, lhs, ones_col, lkb == 0, lkb == NLK - 1)
                        rec = op.tile([128, 1], fp32, tag="rec")
                        nc.vector.reciprocal(rec, ps_n[:, 0:1])
                        o_sb = op.tile([128, DK], bf16, tag="osb")
                        nc.scalar.activation(
                            o_sb, ps_o,
                            mybir.ActivationFunctionType.Copy, scale=rec,
                        )
                        nc.sync.dma_start(
                            out[b, t * c_t + s * 128: t * c_t + (s + 1) * 128, :],
                            o_sb,
                        )
    return nc


_NC_CACHE = {}


def _get_nc(key=("v3",)):
    if key not in _NC_CACHE:
        _NC_CACHE[key] = build_nc()
    return _NC_CACHE[key]


def make_in_maps(inputs):
    """Host prep: cast bf16, pack SBUF-image layouts, shard by batch."""
    import ml_dtypes

    bf16 = ml_dtypes.bfloat16
    f32c = lambda x: np.ascontiguousarray(np.asarray(x), dtype=np.float32)

    # query [B, lq, dq] -> [B, t, p, kc, n]: lq = t*512+n, dq = kc*128+p
    qT = np.ascontiguousarray(
        np.asarray(inputs["query"]).astype(bf16)
        .reshape(B, NCQ, C_T, KCQ, 128).transpose(0, 1, 4, 3, 2)
    )
    kT = np.ascontiguousarray(
        np.asarray(inputs["key"]).astype(bf16)
        .reshape(B, NTK, 512, KCK, 128).transpose(0, 1, 4, 3, 2)
    )
    # W [dk_in, dk_out] -> [p, kc, dk_out]: dk_in = kc*128+p
    def w_img(w, kc):
        return np.ascontiguousarray(
            np.asarray(w).astype(bf16).reshape(kc, 128, -1).transpose(1, 0, 2)
        )

    shared = {
        "Wq": w_img(inputs["Wq"], KCQ),
        "Wk": w_img(inputs["Wk"], KCK),
        "Wv": w_img(inputs["Wv"], KCK),
        "bq": f32c(inputs["bq"]),
        "bv": f32c(inputs["bv"]),
    }
    in_maps = []
    for c in range(N_CORES):
        m = dict(shared)
        m["queryT"] = qT[c * BPC:(c + 1) * BPC]
        m["keyT"] = kT[c * BPC:(c + 1) * BPC]
        in_maps.append(m)
    return in_maps


def kernel(**inputs):
    from concourse.bass_utils import run_bass_kernel_spmd

    nc = _get_nc()
    if not nc.is_finalized():
        nc.finalize()
    in_maps = make_in_maps(inputs)
    res = run_bass_kernel_spmd(nc, in_maps, core_ids=list(range(N_CORES)))
    return np.concatenate(
        [r["out"].astype(np.float32) for r in res.results], axis=0
    )


# revision 20
# speedup vs baseline: 1.0020x; 1.0020x over previous
"""CrossAttention Trainium2 Bass kernel (v7b).

Problem (hardcoded): B=16, Lq=Lk=2048, Dq=768, Dk=1024, fp32.
  q = query @ Wq + bq ; k = key @ Wk + bk ; v = key @ Wv + bv
  out = softmax(q k^T / sqrt(1024)) @ v

Sharding: data-parallel over batch, 2 batches per core on 8 cores.

Math simplifications (exact up to rounding):
  - bk shifts every score row by a per-query constant -> cancels in softmax.
  - bv folds into v (softmax weights sum to 1): v' = k@Wv + bv gives
    out = (sum exp * v') / sum exp directly.
  - scores are bounded (|s|/32 < ~3) so exp() without max-subtraction is safe.

v3 (vs v2): all DMAs use host-prepacked SBUF-image layouts (128 fat
contiguous descriptors per transfer instead of ~1024 thin ones), cutting
DMA-issue serialization at kernel start; weight DMAs issue in phase order
(Wk, Wv first); output is downloaded bf16 and cast to fp32 on the host.
Device work per batch: kT = Wk^T keyT and v = keyT^T Wv + bv resident in
SBUF, then per 512-col Lq tile: qT = Wq^T queryT + bq, scoresT = kT^T qT,
exp(s/32), PV + ones-column row sums, out = PV * (1/sums).
"""

import numpy as np

B, LQ, LK = 16, 2048, 2048
DQ, DK = 768, 1024
N_CORES = 8
BPC = B // N_CORES  # batches per core

KCQ = DQ // 128  # 6 contraction chunks for q projection
KCK = DK // 128  # 8 contraction chunks for k/v projection + scores
NLK = LK // 128  # 16 Lk subtiles of 128
C_T = 512
NCQ = LQ // C_T  # Lq tiles
NTK = LK // 512  # Lk tiles (projection phase)


def build_nc(bpc=BPC, lq=LQ, lk=LK, c_t=C_T):
    import concourse.bass as bass
    import concourse.mybir as mybir
    from concourse import bacc
    import concourse.tile as tile

    fp32 = mybir.dt.float32
    bf16 = mybir.dt.bfloat16
    CS = c_t // 128  # Lq subtiles per attention tile

    nc = bacc.Bacc("TRN2")
    # Host-prepacked SBUF-image layouts: partition dim explicit, per-partition
    # slabs contiguous so every DMA is 128 fat descriptors.
    queryT = nc.dram_tensor(
        "queryT", [bpc, NCQ, 128, KCQ, c_t], bf16, kind="ExternalInput")
    keyT = nc.dram_tensor(
        "keyT", [bpc, NTK, 128, KCK, 512], bf16, kind="ExternalInput")
    Wq = nc.dram_tensor("Wq", [128, KCQ, DK], bf16, kind="ExternalInput")
    Wk = nc.dram_tensor("Wk", [128, KCK, DK], bf16, kind="ExternalInput")
    Wv = nc.dram_tensor("Wv", [128, KCK, DK], bf16, kind="ExternalInput")
    bq = nc.dram_tensor("bq", [DK], fp32, kind="ExternalInput")
    bv = nc.dram_tensor("bv", [DK], fp32, kind="ExternalInput")
    out = nc.dram_tensor("out", [bpc, lq, DK], bf16, kind="ExternalOutput")

    fp8 = mybir.dt.float8e4
    DR = mybir.MatmulPerfMode.DoubleRow

    def mm(ps, lhsT, rhs, start, stop, perf_mode=None):
        nc.tensor.matmul(ps, lhsT, rhs, start=start, stop=stop,
                         perf_mode=perf_mode)

    with tile.TileContext(nc) as tc:
        with (
            tc.tile_pool(name="const", bufs=1) as constp,
            tc.tile_pool(name="w", bufs=1) as wp,
            tc.tile_pool(name="kT", bufs=1) as kTp,
            tc.tile_pool(name="v", bufs=1) as vp,
            tc.tile_pool(name="kstage", bufs=2) as ksp,
            tc.tile_pool(name="qstage", bufs=2) as qsp,
            tc.tile_pool(name="qT", bufs=2) as qTp,
            tc.tile_pool(name="exp", bufs=NLK + 2) as ep,
            tc.tile_pool(name="osb", bufs=2) as op,
            tc.tile_pool(name="ps512", bufs=2, space="PSUM") as ps512,
            tc.tile_pool(name="ps_o", bufs=2, space="PSUM") as ps_op,
            tc.tile_pool(name="ps_n", bufs=2, space="PSUM") as ps_np,
        ):
            # Phase-order DMA issue: Wk halves first (first matmuls), then Wv
            # (needed ~10us in), bv (v evacuation), then Wq/bq (phase A).
            # Interleave Wk and the first key tile so the first accumulation
            # group's operands (kc 0-3 of both) land earliest; the kc 4-7
            # halves stream in during the first matmul group.
            wk_sb = wp.tile([128, KCK, DK], bf16)
            kst0 = ksp.tile([128, KCK, 512], bf16, tag="kst")
            nc.sync.dma_start(wk_sb[:, 0:4, :], Wk[:, 0:4, :])
            nc.sync.dma_start(kst0[:, 0:4, :], keyT[0, 0, :, 0:4, :])
            nc.sync.dma_start(wk_sb[:, 4:8, :], Wk[:, 4:8, :])
            nc.sync.dma_start(kst0[:, 4:8, :], keyT[0, 0, :, 4:8, :])
            wv_sb = wp.tile([128, KCK, DK], bf16)
            nc.sync.dma_start(wv_sb, Wv[:, :, :])
            bv_rep = constp.tile([128, DK], fp32)
            nc.sync.dma_start(bv_rep, bv[None, :].partition_broadcast(128))
            wq_sb = wp.tile([128, KCQ, DK], bf16)
            nc.sync.dma_start(wq_sb, Wq[:, :, :])
            bq_sb = constp.tile([128, KCK], fp32)
            nc.sync.dma_start(bq_sb, bq.rearrange("(c p) -> p c", p=128))
            ones_f32 = constp.tile([128, 4], fp32)
            nc.vector.memset(ones_f32, 1.0)
            ones_col = constp.tile([128, 4], bf16)
            nc.vector.tensor_copy(ones_col, ones_f32)
            # HAM pre-warm: dummy matmuls (outputs never read) during the
            # initial weight-DMA wait open the PE clock gate to K=8/8 before
            # the real matmuls start.
            for _ in range(64):
                ps = ps512.tile([128, 512], fp32, tag="mm512")
                nc.tensor.matmul(ps[0:4, 0:4], ones_col, ones_col,
                                 start=True, stop=True)

            for b in range(bpc):
                # fp8 scores: kT/qT stored fp8e4m3; score matmuls run as
                # DoubleRow pairs (K=256 per matmul, ~2x PE rate).
                kT8_sb = kTp.tile([128, 6, lk], fp8, tag="kT8")   # [dk0:768, lk]
                kT_sb = kTp.tile([128, 2, lk], bf16, tag="kT")    # [dk768:, lk]
                v_sb = vp.tile([128, NLK, DK], bf16, tag="v")      # [lk, dk]

                # ---- Phase B: kT and v (+bv), SBUF resident ----
                for t in range(NTK):
                    if b == 0 and t == 0:
                        kst = kst0
                    else:
                        kst = ksp.tile([128, KCK, 512], bf16, tag="kst")
                        nc.sync.dma_start(kst[:, 0:4, :], keyT[b, t, :, 0:4, :])
                        nc.sync.dma_start(kst[:, 4:8, :], keyT[b, t, :, 4:8, :])
                    for mc in range(KCK):
                        ps = ps512.tile([128, 512], fp32, tag="mm512")
                        for kc in range(KCK):
                            mm(ps, wk_sb[:, kc, mc * 128:(mc + 1) * 128],
                               kst[:, kc, :], kc == 0, kc == KCK - 1)
                        if mc < 6:
                            nc.vector.tensor_copy(
                                kT8_sb[:, mc, t * 512:(t + 1) * 512], ps
                            )
                        else:
                            nc.vector.tensor_copy(
                                kT_sb[:, mc - 6, t * 512:(t + 1) * 512], ps
                            )
                    for s in range(4):
                        for dk in range(2):
                            ps = ps_op.tile([128, DK], fp32, tag="pv")
                            for kc in range(KCK):
                                mm(ps[:, 0:512],
                                   kst[:, kc, s * 128:(s + 1) * 128],
                                   wv_sb[:, kc, dk * 512:(dk + 1) * 512],
                                   kc == 0, kc == KCK - 1)
                            nc.vector.tensor_add(
                                v_sb[:, t * 4 + s, dk * 512:(dk + 1) * 512],
                                ps[:, 0:512],
                                bv_rep[:, dk * 512:(dk + 1) * 512],
                            )

                # ---- Phase A+C fused per Lq tile ----
                for t in range(NCQ):
                    qst = qsp.tile([128, KCQ, c_t], bf16, tag="qst")
                    nc.sync.dma_start(qst, queryT[b, t])
                    qT8 = qTp.tile([128, 6, c_t], fp8, tag="qT8")
                    qTt = qTp.tile([128, 2, c_t], bf16, tag="qTt")
                    for mc in range(KCK):
                        ps = ps512.tile([128, c_t], fp32, tag="mm512")
                        for kc in range(KCQ):
                            mm(ps, wq_sb[:, kc, mc * 128:(mc + 1) * 128],
                               qst[:, kc, :], kc == 0, kc == KCQ - 1)
                        if mc < 6:
                            nc.vector.tensor_scalar_add(
                                qT8[:, mc, :], ps, bq_sb[:, mc:mc + 1]
                            )
                        else:
                            nc.vector.tensor_scalar_add(
                                qTt[:, mc - 6, :], ps, bq_sb[:, mc:mc + 1]
                            )
                    exps = []
                    for lkb in range(NLK):
                        ps_s = ps512.tile([128, c_t], fp32, tag="mm512")
                        lks = slice(lkb * 128, (lkb + 1) * 128)
                        for j in range(3):
                            mm(ps_s, kT8_sb[:, 2 * j:2 * j + 2, lks],
                               qT8[:, 2 * j:2 * j + 2, :],
                               j == 0, False, perf_mode=DR)
                        for kc in range(2):
                            mm(ps_s, kT_sb[:, kc, lks],
                               qTt[:, kc, :], False, kc == 1)
                        ex = ep.tile([128, c_t], bf16, tag="exp")
                        nc.scalar.activation(
                            ex, ps_s, mybir.ActivationFunctionType.Exp,
                            scale=1.0 / 32.0,
                        )
                        exps.append(ex)
                    for s in range(CS):
                        ps_o = ps_op.tile([128, DK], fp32, tag="pv")
                        ps_n = ps_np.tile([128, 4], fp32, tag="sum")
                        for lkb in range(NLK):
                            lhs = exps[lkb][:, s * 128:(s + 1) * 128]
                            for dk in range(2):
                                mm(ps_o[:, dk * 512:(dk + 1) * 512], lhs,
                                   v_sb[:, lkb, dk * 512:(dk + 1) * 512],
                                   lkb == 0, lkb == NLK - 1)
                            mm(ps_n, lhs, ones_col, lkb == 0, lkb == NLK - 1)
                        rec = op.tile([128, 1], fp32, tag="rec")
                        nc.vector.reciprocal(rec, ps_n[:, 0:1])
                        o_sb = op.tile([128, DK], bf16, tag="osb")
                        nc.scalar.activation(
                            o_sb, ps_o,
                            mybir.ActivationFunctionType.Copy, scale=rec,
                        )
                        nc.sync.dma_start(
                            out[b, t * c_t + s * 128: t * c_t + (s + 1) * 128, :],
                            o_sb,
                        )
    return nc


_NC_CACHE = {}


def _get_nc(key=("v3",)):
    if key not in _NC_CACHE:
        _NC_CACHE[key] = build_nc()
    return _NC_CACHE[key]


def make_in_maps(inputs):
    """Host prep: cast bf16, pack SBUF-image layouts, shard by batch."""
    import ml_dtypes

    bf16 = ml_dtypes.bfloat16
    f32c = lambda x: np.ascontiguousarray(np.asarray(x), dtype=np.float32)

    # query [B, lq, dq] -> [B, t, p, kc, n]: lq = t*512+n, dq = kc*128+p
    qT = np.ascontiguousarray(
        np.asarray(inputs["query"]).astype(bf16)
        .reshape(B, NCQ, C_T, KCQ, 128).transpose(0, 1, 4, 3, 2)
    )
    kT = np.ascontiguousarray(
        np.asarray(inputs["key"]).astype(bf16)
        .reshape(B, NTK, 512, KCK, 128).transpose(0, 1, 4, 3, 2)
    )
    # W [dk_in, dk_out] -> [p, kc, dk_out]: dk_in = kc*128+p
    def w_img(w, kc):
        return np.ascontiguousarray(
            np.asarray(w).astype(bf16).reshape(kc, 128, -1).transpose(1, 0, 2)
        )

    shared = {
        "Wq": w_img(inputs["Wq"], KCQ),
        "Wk": w_img(inputs["Wk"], KCK),
        "Wv": w_img(inputs["Wv"], KCK),
        "bq": f32c(inputs["bq"]),
        "bv": f32c(inputs["bv"]),
    }
    in_maps = []
    for c in range(N_CORES):
        m = dict(shared)
        m["queryT"] = qT[c * BPC:(c + 1) * BPC]
        m["keyT"] = kT[c * BPC:(c + 1) * BPC]
        in_maps.append(m)
    return in_maps


def kernel(**inputs):
    from concourse.bass_utils import run_bass_kernel_spmd

    nc = _get_nc()
    if not nc.is_finalized():
        nc.finalize()
    in_maps = make_in_maps(inputs)
    res = run_bass_kernel_spmd(nc, in_maps, core_ids=list(range(N_CORES)))
    return np.concatenate(
        [r["out"].astype(np.float32) for r in res.results], axis=0
    )
